# revision 17
# baseline (speedup 1.0000x reference)
"""NewtonNet embedding kernel for 8 TRN2 NeuronCores (Bass/Tile).

Strategy (graph/data parallel on edges, per the sharding hint):
  - Edges (sorted by src) are sharded 8 ways at segment boundaries; each core
    owns the contiguous node range its edges cover.  Within a core, owned
    nodes are sorted by degree; the edge for node-slot `128*t + p` (round r)
    sits at partition p of edge block (tile t, round r), so segment sums are
    per-partition accumulates and the src-side gather is the identity.
  - All matmul operands are fp16 (1 cy/row on the PE vs 4 for fp32) with fp32
    PSUM accumulation.  The edge pipeline runs transposed ([c, e] layout):
    DijT comes straight out of the radial matmul, the dst-gather returns
    transposed rows, and mijT feeds the edge MLPs with no PE transpose.
  - Per-layer a-MLP output is AllGathered (fp16) and parked in SBUF in the
    dma_gather SBUF-source stripe layout; dst rows are fetched with
    transposed SBUF-source gathers (256B elements).
  - xi segment-sum runs on the PE (mijT.T @ I accumulated in PSUM per tile).
  - fi/phi segment sums are per-partition scalar*tensor accumulates split
    between DVE (stt) and ACT (scaled copy) + DVE add.
"""

import os
import sys

import numpy as np

sys.path.insert(0, "/opt/trn_rl_repo")

N_CORES = 8
DIM = 128
NLAYERS = 3
NBASIS = 16
CUTOFF = 5.0
CHUNK_BLOCKS = 8  # edge blocks per dma_gather chunk (dma_gather caps at 1024 idxs)

# ---------------------------------------------------------------------------
# Host-side preprocessing
# ---------------------------------------------------------------------------


def host_prep(inputs):
    species = np.asarray(inputs["species"])
    src = np.asarray(inputs["edge_src"])
    dst = np.asarray(inputs["edge_dst"])
    d = np.asarray(inputs["distances"]).astype(np.float32)
    vec = np.asarray(inputs["vec"]).astype(np.float32)
    sw = np.asarray(inputs["switch"]).astype(np.float32)
    E = src.shape[0]
    N = species.shape[0]

    cuts = [0]
    for c in range(1, N_CORES):
        k = c * E // N_CORES
        while k < E and src[k] == src[k - 1]:
            k += 1
        cuts.append(k)
    cuts.append(E)
    nb = [0]
    for c in range(1, N_CORES):
        nb.append(int(src[cuts[c]]) if cuts[c] < E else N)
    nb.append(N)

    deg = np.bincount(src, minlength=N)
    cores = []
    for c in range(N_CORES):
        lo, hi = nb[c], nb[c + 1]
        nodes = np.arange(lo, hi)
        order = np.argsort(-deg[lo:hi], kind="stable")
        perm = nodes[order]
        cores.append(dict(perm=perm, deg=deg[perm]))

    NT = max((len(c["perm"]) + 127) // 128 for c in cores)
    n_pad = NT * 128

    R = []
    for t in range(NT):
        r = 0
        for c in cores:
            dg = c["deg"]
            if 128 * t < len(dg):
                r = max(r, int(dg[128 * t]))
        R.append(max(r, 1))
    B = sum(R)
    E_dev = 128 * B

    owner = np.zeros(N, np.int32)
    slot = np.zeros(N, np.int32)
    for ci, c in enumerate(cores):
        owner[c["perm"]] = ci
        slot[c["perm"]] = np.arange(len(c["perm"]))

    centers = np.linspace(0.0, CUTOFF, NBASIS).astype(np.float32)
    eta = np.float32((NBASIS / CUTOFF) ** 2)
    estart = np.zeros(N + 1, np.int64)
    np.cumsum(deg, out=estart[1:])

    per_core = []
    for ci, c in enumerate(cores):
        perm, dgs = c["perm"], c["deg"]
        n_own = len(perm)
        # device edge id for (slot, round) -> global edge index (or -1 dummy)
        eid = np.full(E_dev, -1, np.int64)
        off = 0
        for t in range(NT):
            s0 = 128 * t
            s1 = min(s0 + 128, n_own)
            if s1 > s0:
                pp = np.arange(s0, s1)
                dg_t = dgs[pp]
                base = estart[perm[pp]]
                for r in range(R[t]):
                    b = off + r
                    m = r < dg_t
                    idx = 128 * b + (pp - s0)
                    eid[idx[m]] = base[m] + r
            off += R[t]
        valid = eid >= 0
        ge = np.where(valid, eid, 0)

        de = d[ge]
        swe = np.where(valid, sw[ge], 0.0).astype(np.float32)
        rbsw = np.zeros((17, E_dev), np.float16)
        rbsw[:16] = (
            np.exp(-eta * (de[None, :] - centers[:, None]) ** 2) * swe[None, :]
        ).astype(np.float16)
        rbsw[16] = swe.astype(np.float16)

        dire = vec[ge] / de[:, None] * swe[:, None]

        dsti = (owner[dst[ge]].astype(np.int64) * n_pad + slot[dst[ge]]).astype(
            np.uint16
        )
        dsti[~valid] = 0

        sw_dev = swe.reshape(B, 128)
        swsum = np.zeros((128, NT), np.float32)
        o = 0
        for t in range(NT):
            swsum[:, t] = sw_dev[o : o + R[t]].sum(axis=0)
            o += R[t]

        swcol = np.ascontiguousarray(sw_dev.T)  # [128, B]
        dirsw = np.ascontiguousarray(
            dire.reshape(B, 128, 3).transpose(1, 0, 2).reshape(128, B * 3)
        )
        dsti_w = np.tile(dsti.reshape(-1, 16).T, (8, 1))  # replicated for 8 Q7 cores
        spi = np.zeros(n_pad, np.int16)
        spi[:n_own] = species[perm]
        spi_w = np.tile(spi.reshape(-1, 16).T, (8, 1))

        per_core.append(
            dict(
                rbsw=rbsw,
                dirsw=dirsw,
                swcol=swcol,
                swsum=swsum,
                dsti_w=dsti_w,
                spi_w=spi_w,
                perm=perm,
                n_own=n_own,
            )
        )

    meta = dict(NT=NT, n_pad=n_pad, R=R, B=B, E_dev=E_dev)
    return meta, per_core


def pack_weights(inputs):
    """Pack weight matrices into one fp16 [NW,128,128] tensor + bias tensors."""
    W = {k: np.asarray(v).astype(np.float32) for k, v in inputs.items()}
    mats = []
    widx = {}

    def add(name, m):
        widx[name] = len(mats)
        p = np.zeros((128, 128), np.float16)
        p[: m.shape[0], : m.shape[1]] = m.astype(np.float16)
        mats.append(p)

    for l in range(NLAYERS):
        for nm in ["a", "F", "f", "r", "R", "u"]:
            add(f"{nm}_W1_{l}", W[f"{nm}_W1"][l])
            add(f"{nm}_W2_{l}", W[f"{nm}_W2"][l])
        add(
            f"radial_Wb_{l}",
            np.concatenate([W["radial_W"][l], W["radial_b"][l][None]], 0),
        )
    spw = np.zeros((128, 128), np.float32)
    spw[:100] = W["species_W"]
    wts = np.stack(mats)  # [NW,128,128] fp16

    # bias columns [128, NC] fp32:
    #   cols 6l+j          : b1 of (a,F,f,r,R,u) layer l
    #   cols 18+l          : F_b2 layer l (broadcast scalar)
    #   cols 21+l          : a_b2 layer l
    bcols = np.zeros((128, 6 * NLAYERS + 2 * NLAYERS), np.float32)
    for l in range(NLAYERS):
        for j, nm in enumerate(["a", "F", "f", "r", "R", "u"]):
            bcols[:, 6 * l + j] = W[f"{nm}_b1"][l]
        bcols[:, 6 * NLAYERS + l] = W["F_b2"][l][0]
        bcols[:, 7 * NLAYERS + l] = W["a_b2"][l]

    # broadcast b2 tiles [NB,128,128] fp16: 0=species_b; per-layer f,r,R,u
    bidx = {"species": 0}
    bl = [np.tile(W["species_b"][None], (128, 1)).astype(np.float16)]
    for nm in ["f", "r", "R", "u"]:
        for l in range(NLAYERS):
            bidx[f"{nm}_{l}"] = len(bl)
            bl.append(np.tile(W[f"{nm}_b2"][l][None], (128, 1)).astype(np.float16))
    bbc = np.stack(bl)
    return wts, widx, bcols, bbc, bidx, spw


# ---------------------------------------------------------------------------
# Device kernel builder
# ---------------------------------------------------------------------------


def build_nc(meta, widx, bidx, NW, NB_bbc, no_cc=False, no_gather=False, nlayers=NLAYERS):
    import concourse.bass as bass
    import concourse.bacc as bacc
    import concourse.mybir as mybir
    import concourse.tile as tile
    from concourse.masks import make_identity

    NT, n_pad, R, B, E_dev = meta["NT"], meta["n_pad"], meta["R"], meta["B"], meta["E_dev"]
    NS = N_CORES * n_pad // 128  # stripes in the SBUF gather source
    f32 = mybir.dt.float32
    f16 = mybir.dt.float16
    i16 = mybir.dt.int16
    Alu = mybir.AluOpType
    Act = mybir.ActivationFunctionType

    nc = bacc.Bacc("TRN2", target_bir_lowering=False, debug=False, num_devices=N_CORES)

    # ---- I/O ----
    wts_d = nc.dram_tensor("wts", [NW, 128, 128], f16, kind="ExternalInput")
    spw_d = nc.dram_tensor("spw", [128, 128], f32, kind="ExternalInput")
    bcols_d = nc.dram_tensor("bcols", [128, 8 * NLAYERS], f32, kind="ExternalInput")
    bbc_d = nc.dram_tensor("bbc", [NB_bbc, 128, 128], f16, kind="ExternalInput")
    rbsw_d = nc.dram_tensor("rbsw", [17, E_dev], f16, kind="ExternalInput")
    dsti_d = nc.dram_tensor("dsti", [128, E_dev // 16], mybir.dt.uint16, kind="ExternalInput")
    spi_d = nc.dram_tensor("spi", [128, n_pad // 16], i16, kind="ExternalInput")
    swcol_d = nc.dram_tensor("swcol", [128, B], f32, kind="ExternalInput")
    dirsw_d = nc.dram_tensor("dirsw", [128, 3 * B], f32, kind="ExternalInput")
    swsum_d = nc.dram_tensor("swsum", [128, NT], f32, kind="ExternalInput")
    xi_out_d = nc.dram_tensor("xi_out", [n_pad, 128], f32, kind="ExternalOutput")

    ai_bounce = nc.dram_tensor("ai_bounce", [128, n_pad], f16)
    ai_all_sh = nc.dram_tensor("ai_all_sh", [N_CORES * 128, n_pad], f16, addr_space="Shared")

    # ---- persistent SBUF ----
    wts_sb = nc.alloc_sbuf_tensor("wts_sb", [128, NW * 128], f16)
    bcols_sb = nc.alloc_sbuf_tensor("bcols_sb", [128, 8 * NLAYERS], f32)
    bbc_sb = nc.alloc_sbuf_tensor("bbc_sb", [128, NB_bbc * 128], f16)
    swcol_sb = nc.alloc_sbuf_tensor("swcol_sb", [128, B], f32)
    dirsw_sb = nc.alloc_sbuf_tensor("dirsw_sb", [128, 3 * B], f32)
    swsum_sb = nc.alloc_sbuf_tensor("swsum_sb", [128, NT], f32)
    dsti_sb = nc.alloc_sbuf_tensor("dsti_sb", [128, E_dev // 16], mybir.dt.uint16)
    spi_sb = nc.alloc_sbuf_tensor("spi_sb", [128, n_pad // 16], i16)
    ident_sb = nc.alloc_sbuf_tensor("ident_sb", [128, 128], f16)

    xi_sb = nc.alloc_sbuf_tensor("xi_sb", [128, NT * 128], f32)    # node-major state
    xi16_sb = nc.alloc_sbuf_tensor("xi16_sb", [128, NT * 128], f16)
    xiT_sb = nc.alloc_sbuf_tensor("xiT_sb", [128, n_pad], f16)     # [c, node]
    aiT_sb = nc.alloc_sbuf_tensor("aiT_sb", [128, n_pad], f16)     # [c, node]
    aiallT_sb = nc.alloc_sbuf_tensor("aiallT_sb", [128, N_CORES * n_pad], f16)  # [c, global node]
    fi_sb = nc.alloc_sbuf_tensor("fi_sb", [128, NT * 3 * 128], f16)
    di_sb = nc.alloc_sbuf_tensor("di_sb", [128, NT * 3 * 128], f16)
    phi_sb = nc.alloc_sbuf_tensor("phi_sb", [128, NT * 128], f16)
    Rout_sb = nc.alloc_sbuf_tensor("Rout_sb", [128, NT * 128], f16)
    scal_sb = nc.alloc_sbuf_tensor("scal_sb", [128, NT * 128], f16)
    tmp16_sb = nc.alloc_sbuf_tensor("tmp16_sb", [128, NT * 128], f16)
    fvs_sb = nc.alloc_sbuf_tensor("fvs_sb", [128, 12], f32)
    fvs3_sb = nc.alloc_sbuf_tensor("fvs3_sb", [128, 3], f32)

    def W(name):
        m = widx[name]
        return wts_sb[:, 128 * m : 128 * (m + 1)]

    def Bb(name):
        m = bidx[name]
        return bbc_sb[:, 128 * m : 128 * (m + 1)]

    def b1col(l, nm):
        j = 6 * l + ["a", "F", "f", "r", "R", "u"].index(nm)
        return bcols_sb[:, j : j + 1]

    def ap3(ap2d, base, gw):
        """[128, gw, 128] view of contiguous cols [base, base+gw*128)."""
        a = ap2d[:, base : base + gw * 128]
        return bass.AP(
            tensor=a.tensor, offset=a.offset, ap=[a.ap[0], [128, gw], [1, 128]]
        )

    def rep3(ap2d, base, gw):
        """[128, gw, 128] broadcast of cols [base, base+128) repeated gw times."""
        a = ap2d[:, base : base + 128]
        return bass.AP(tensor=a.tensor, offset=a.offset, ap=[a.ap[0], [0, gw], [1, 128]])

    # tile index of each block, block ranges per tile
    tile_off = []
    o = 0
    for t in range(NT):
        tile_off.append(o)
        o += R[t]
    n_chunks = (B + CHUNK_BLOCKS - 1) // CHUNK_BLOCKS

    with tile.TileContext(nc) as tc:
        with (
            tc.tile_pool(name="psA", bufs=2, space="PSUM") as psA,  # DijT / transposes
            tc.tile_pool(name="psC", bufs=2, space="PSUM") as psC,  # mm1 h1
            tc.tile_pool(name="psD", bufs=2, space="PSUM") as psD,  # mm2 outs
            tc.tile_pool(name="psX", bufs=1, space="PSUM") as psX,  # xi accum
            tc.tile_pool(name="sbg", bufs=5) as sbg,  # adT gather chunks
            tc.tile_pool(name="sbr", bufs=5) as sbr,  # rbsw chunks
            tc.tile_pool(name="sbt", bufs=2) as sbt,  # tmp = adT*aiT
            tc.tile_pool(name="sbm", bufs=2) as sbm,  # mijT
            tc.tile_pool(name="sbh", bufs=4) as sbh,  # h1s tiles
            tc.tile_pool(name="sbp", bufs=3) as sbp,  # ACT fi products
            tc.tile_pool(name="sbv", bufs=2) as sbv,  # fv smalls
        ):
            # ---- load resident data ----
            nc.sync.dma_start(
                wts_sb[:].rearrange("p (m n) -> p m n", m=NW),
                wts_d[:].rearrange("m p n -> p m n"),
            )
            nc.sync.dma_start(bcols_sb[:], bcols_d[:])
            nc.sync.dma_start(
                bbc_sb[:].rearrange("p (m n) -> p m n", m=NB_bbc),
                bbc_d[:].rearrange("m p n -> p m n"),
            )
            nc.sync.dma_start(swcol_sb[:], swcol_d[:])
            nc.sync.dma_start(dirsw_sb[:], dirsw_d[:])
            nc.sync.dma_start(swsum_sb[:], swsum_d[:])
            nc.sync.dma_start(dsti_sb[:], dsti_d[:])
            nc.sync.dma_start(spi_sb[:], spi_d[:])
            make_identity(nc, ident_sb[:])

            # ---- xi0 = species_W[species] + species_b ----
            for g0 in range(0, NT, 8):
                gn = min(8, NT - g0)
                nc.gpsimd.dma_gather(
                    out_ap=xi_sb[:, 128 * g0 : 128 * (g0 + gn)].rearrange(
                        "p (t c) -> p t c", t=gn
                    ),
                    in_ap=spw_d[:],
                    idxs_ap=spi_sb[:, 8 * g0 : 8 * (g0 + gn)],
                    num_idxs=128 * gn,
                    num_idxs_reg=128 * gn,
                    elem_size=128,
                )
            nc.vector.tensor_tensor(
                out=ap3(xi_sb[:], 0, NT),
                in0=ap3(xi_sb[:], 0, NT),
                in1=rep3(bbc_sb[:], 128 * bidx["species"], NT),
                op=Alu.add,
            )
            nc.vector.memset(fi_sb[:], 0.0)

            def transpose_pass():
                """xi (f32 node-major) -> xi16 -> xiT (f16 [c, node])."""
                nc.scalar.copy(out=xi16_sb[:], in_=xi_sb[:])
                for t in range(NT):
                    ps = psC.tile([128, 512], f16, tag="h1")
                    nc.tensor.transpose(
                        out=ps[:, 0:128],
                        in_=xi16_sb[:, 128 * t : 128 * (t + 1)],
                        identity=ident_sb[:],
                    )
                    nc.scalar.copy(
                        out=xiT_sb[:, 128 * t : 128 * (t + 1)], in_=ps[:, 0:128]
                    )

            # =========================== layers ===========================
            for l in range(nlayers):
                transpose_pass()

                # ---- a-MLP (wide, transposed output) ----
                for c0 in range(0, n_pad, 512):
                    w = min(512, n_pad - c0)
                    h1p = psC.tile([128, 512], f32, tag="h1")
                    nc.tensor.matmul(
                        h1p[:, :w], lhsT=W(f"a_W1_{l}"), rhs=xiT_sb[:, c0 : c0 + w],
                        start=True, stop=True,
                    )
                    h1s = sbh.tile([128, 512], f16, tag="h1s")
                    nc.scalar.activation(
                        out=h1s[:, :w], in_=h1p[:, :w], func=Act.Silu, bias=b1col(l, "a")
                    )
                    pa = psD.tile([128, 512], f32, tag="mm2")
                    nc.tensor.matmul(
                        pa[:, :w], lhsT=W(f"a_W2_{l}"), rhs=h1s[:, :w],
                        start=True, stop=True,
                    )
                    nc.scalar.add(
                        out=aiT_sb[:, c0 : c0 + w], in_=pa[:, :w],
                        add=bcols_sb[:, 7 * NLAYERS + l : 7 * NLAYERS + l + 1],
                    )
                nc.sync.dma_start(ai_bounce[:], aiT_sb[:])
                if no_cc:
                    for _ci in range(N_CORES):
                        nc.sync.dma_start(
                            ai_all_sh[_ci * 128 : (_ci + 1) * 128, :], ai_bounce[:]
                        )
                else:
                    nc.gpsimd.collective_compute(
                        "AllGather",
                        Alu.bypass,
                        replica_groups=[list(range(N_CORES))],
                        ins=[ai_bounce.ap().opt()],
                        outs=[ai_all_sh.ap().opt()],
                    )
                nc.sync.dma_start(
                    aiallT_sb[:].rearrange("p (g n) -> p g n", g=N_CORES),
                    ai_all_sh[:].rearrange("(g p) n -> p g n", p=128),
                )
                if l > 0:
                    nc.vector.memset(phi_sb[:], 0.0)

                # ---- edge phase ----
                chunk_tiles = {}

                def issue_chunk(ci):
                    b0 = ci * CHUNK_BLOCKS
                    nb_ = min(CHUNK_BLOCKS, B - b0)
                    ad = sbg.tile([128, CHUNK_BLOCKS * 128], f16, tag="ad")
                    if no_gather:
                        nc.vector.memset(ad[:], 1.0)
                    else:
                        nc.gpsimd.indirect_copy(
                            out=ad[:, : nb_ * 128].rearrange(
                                "p (e o) -> p e o", o=1
                            ),
                            data=aiallT_sb[:].rearrange("p (n o) -> p n o", o=1),
                            idxs=dsti_sb[:, 8 * b0 : 8 * (b0 + nb_)],
                            i_know_ap_gather_is_preferred=True,
                        )
                    rb = sbr.tile([17, CHUNK_BLOCKS * 128], f16, tag="rbsw")
                    nc.sync.dma_start(
                        rb[:, : nb_ * 128], rbsw_d[:, 128 * b0 : 128 * (b0 + nb_)]
                    )
                    chunk_tiles[ci] = (ad, rb, b0)

                def get_chunk(ci):
                    if ci not in chunk_tiles:
                        issue_chunk(ci)
                    for cn in (ci + 1, ci + 2, ci + 3):
                        if cn < n_chunks and cn not in chunk_tiles:
                            issue_chunk(cn)
                    return chunk_tiles[ci]

                # ---- two-stage pipelined group loop ----
                groups = []
                for t in range(NT):
                    r0 = 0
                    while r0 < R[t]:
                        b0 = tile_off[t] + r0
                        gw = min(4, R[t] - r0, CHUNK_BLOCKS - b0 % CHUNK_BLOCKS)
                        groups.append((t, b0, gw, r0 == 0, r0 + gw >= R[t]))
                        r0 += gw
                mlps = ["F", "f"] + (["r"] if l > 0 else [])
                st = {}
                tile_ps = {}

                def FRONT(gi):
                    t, b0, gw, first, last = groups[gi]
                    ci = b0 // CHUNK_BLOCKS
                    ad, rb, cb0 = get_chunk(ci)
                    w = gw * 128
                    boff = (b0 - cb0) * 128
                    # DijT [c, e] straight from the radial matmul
                    dij = psA.tile([128, 512], f32, tag="dij")
                    nc.tensor.matmul(
                        dij[:, :w], lhsT=W(f"radial_Wb_{l}")[:17, :],
                        rhs=rb[:17, boff : boff + w], start=True, stop=True,
                    )
                    # tmpT = adT * aiT(bcast over blocks)  (DVE 2x fp16)
                    tmpt = sbt.tile([128, 512], f16, tag="tmp")
                    nc.vector.tensor_tensor(
                        out=ap3(tmpt[:], 0, gw), in0=ap3(ad[:], boff, gw),
                        in1=rep3(aiT_sb[:], 128 * t, gw), op=Alu.mult,
                    )
                    # DijT PSUM -> SBUF fp16 (ACT)
                    sdij = sbt.tile([128, 512], f16, tag="sdij")
                    nc.scalar.copy(out=sdij[:, :w], in_=dij[:, :w])
                    # mijT = tmpT * DijT  (DVE 2x fp16)
                    mijT = sbm.tile([128, 512], f16, tag="mijT", bufs=3)
                    nc.vector.tensor_tensor(
                        out=mijT[:, :w], in0=tmpt[:, :w], in1=sdij[:, :w],
                        op=Alu.mult,
                    )
                    st[gi] = mijT

                def BACK(gi):
                    t, b0, gw, first, last = groups[gi]
                    w = gw * 128
                    mijT = st.pop(gi)
                    if first:
                        tile_ps[t] = psX.tile(
                            [128, 128], f32, tag="xiacc", bufs=1, name=f"xiacc{l}_{t}"
                        )
                        nc.vector.memset(fvs_sb[:], 0.0)
                    psxt = tile_ps[t]
                    # xi segment sum on PE: psxt += mijT_j.T @ I
                    for j in range(gw):
                        nc.tensor.matmul(
                            psxt[:, :], lhsT=mijT[:, 128 * j : 128 * (j + 1)],
                            rhs=ident_sb[:],
                            start=first and j == 0, stop=last and j == gw - 1,
                            skip_group_check=True,
                        )
                    # edge MLP hidden layers
                    h1 = {}
                    for nm in mlps:
                        hp = psC.tile([128, 512], f32, tag="h1")
                        nc.tensor.matmul(
                            hp[:, :w], lhsT=W(f"{nm}_W1_{l}"), rhs=mijT[:, :w],
                            start=True, stop=True,
                        )
                        hs = sbh.tile([128, 512], f16, tag="h1s")
                        nc.scalar.activation(
                            out=hs[:, :w], in_=hp[:, :w], func=Act.Silu,
                            bias=b1col(l, nm),
                        )
                        h1[nm] = hs

                    # F head: one column per block
                    pf = psD.tile([128, 4], f32, tag="mm2")
                    for j in range(gw):
                        nc.tensor.matmul(
                            pf[:, j : j + 1],
                            lhsT=h1["F"][:, 128 * j : 128 * (j + 1)],
                            rhs=W(f"F_W2_{l}")[:, 0:1],
                            start=True, stop=True,
                        )
                    # fv = (F + F_b2) * dirsw    [128, gw*3], block-major
                    fvb = sbv.tile([128, 4], f32, tag="fvb")
                    nc.vector.tensor_scalar(
                        out=fvb[:, :gw], in0=pf[:, :gw],
                        scalar1=bcols_sb[:, 6 * NLAYERS + l : 6 * NLAYERS + l + 1],
                        scalar2=None, op0=Alu.add,
                    )
                    fv12 = sbv.tile([128, 12], f32, tag="fv12")
                    d0 = dirsw_sb[:, 3 * b0 : 3 * (b0 + gw)]
                    nc.vector.tensor_tensor(
                        out=bass.AP(tensor=fv12[:].tensor, offset=fv12[:].offset,
                                    ap=[fv12[:].ap[0], [3, gw], [1, 3]]),
                        in0=bass.AP(tensor=d0.tensor, offset=d0.offset,
                                    ap=[d0.ap[0], [3, gw], [1, 3]]),
                        in1=bass.AP(tensor=fvb[:].tensor, offset=fvb[:].offset,
                                    ap=[fvb[:].ap[0], [1, gw], [0, 3]]),
                        op=Alu.mult,
                    )
                    nc.vector.tensor_tensor(
                        out=fvs_sb[:, : 3 * gw], in0=fvs_sb[:, : 3 * gw],
                        in1=fv12[:, : 3 * gw], op=Alu.add,
                    )

                    # f head + fi accumulation
                    pd_f = psD.tile([128, 512], f32, tag="mm2")
                    for j in range(gw):
                        nc.tensor.matmul(
                            pd_f[:, 128 * j : 128 * (j + 1)],
                            lhsT=h1["f"][:, 128 * j : 128 * (j + 1)],
                            rhs=W(f"f_W2_{l}"),
                            start=True, stop=True,
                        )
                    for j in range(gw):
                        fo = pd_f[:, 128 * j : 128 * (j + 1)]
                        for k in range(2):
                            fslice = fi_sb[
                                :, (3 * t + k) * 128 : (3 * t + k + 1) * 128
                            ]
                            nc.vector.scalar_tensor_tensor(
                                out=fslice, in0=fo,
                                scalar=fv12[:, 3 * j + k : 3 * j + k + 1],
                                in1=fslice, op0=Alu.mult, op1=Alu.add,
                            )
                        # k=2 via ACT scaled copy + DVE add
                        prod = sbp.tile([128, 128], f16, tag="prod")
                        nc.scalar.activation(
                            out=prod[:], in_=fo, func=Act.Copy,
                            scale=fv12[:, 3 * j + 2 : 3 * j + 3],
                        )
                        fslice = fi_sb[:, (3 * t + 2) * 128 : (3 * t + 3) * 128]
                        nc.vector.tensor_tensor(
                            out=fslice, in0=fslice, in1=prod[:], op=Alu.add
                        )

                    # r head + phi accumulation
                    if l > 0:
                        pd_r = psD.tile([128, 512], f32, tag="mm2")
                        for j in range(gw):
                            nc.tensor.matmul(
                                pd_r[:, 128 * j : 128 * (j + 1)],
                                lhsT=h1["r"][:, 128 * j : 128 * (j + 1)],
                                rhs=W(f"r_W2_{l}"),
                                start=True, stop=True,
                            )
                        pslice = phi_sb[:, 128 * t : 128 * (t + 1)]
                        for j in range(gw):
                            nc.vector.scalar_tensor_tensor(
                                out=pslice,
                                in0=pd_r[:, 128 * j : 128 * (j + 1)],
                                scalar=swcol_sb[:, b0 + j : b0 + j + 1],
                                in1=pslice, op0=Alu.mult, op1=Alu.add,
                            )

                    if last:
                        # ---- tile end folds ----
                        xt = xi_sb[:, 128 * t : 128 * (t + 1)]
                        nc.vector.tensor_tensor(out=xt, in0=xt, in1=psxt[:, :], op=Alu.add)
                        nc.vector.tensor_reduce(
                            out=fvs3_sb[:],
                            in_=bass.AP(tensor=fvs_sb[:].tensor, offset=fvs_sb[:].offset,
                                        ap=[fvs_sb[:].ap[0], [1, 3], [3, 4]]),
                            axis=mybir.AxisListType.X,
                            op=Alu.add,
                        )
                        for k in range(3):
                            fslice = fi_sb[:, (3 * t + k) * 128 : (3 * t + k + 1) * 128]
                            nc.vector.scalar_tensor_tensor(
                                out=fslice, in0=Bb(f"f_{l}"), scalar=fvs3_sb[:, k : k + 1],
                                in1=fslice, op0=Alu.mult, op1=Alu.add,
                            )
                        if l > 0:
                            pslice = phi_sb[:, 128 * t : 128 * (t + 1)]
                            nc.vector.scalar_tensor_tensor(
                                out=pslice, in0=Bb(f"r_{l}"), scalar=swsum_sb[:, t : t + 1],
                                in1=pslice, op0=Alu.mult, op1=Alu.add,
                            )
                        del tile_ps[t]

                for gi in range(len(groups) + 1):
                    if gi < len(groups):
                        FRONT(gi)
                    if gi >= 1:
                        BACK(gi - 1)

                # ---- node phase ----
                transpose_pass()  # xi is now xi_mid

                # R-MLP and u-MLP (hidden wide; mm2 per tile, node-major out)
                pd_u = {}
                for nm, dest in (("R", "Rout"), ("u", None)):
                    for c0 in range(0, n_pad, 512):
                        w = min(512, n_pad - c0)
                        h1p = psC.tile([128, 512], f32, tag="h1")
                        nc.tensor.matmul(
                            h1p[:, :w], lhsT=W(f"{nm}_W1_{l}"),
                            rhs=xiT_sb[:, c0 : c0 + w], start=True, stop=True,
                        )
                        h1s = sbh.tile([128, 512], f16, tag="h1s")
                        nc.scalar.activation(
                            out=h1s[:, :w], in_=h1p[:, :w], func=Act.Silu,
                            bias=b1col(l, nm),
                        )
                        pd = psD.tile([128, 512], f32, tag="mm2")
                        for j in range(w // 128):
                            nc.tensor.matmul(
                                pd[:, 128 * j : 128 * (j + 1)],
                                lhsT=h1s[:, 128 * j : 128 * (j + 1)],
                                rhs=W(f"{nm}_W2_{l}"),
                                start=True, stop=True,
                            )
                        for j in range(w // 128):
                            tt = (c0 + 128 * j) // 128
                            if dest == "Rout":
                                nc.vector.tensor_tensor(
                                    out=Rout_sb[:, 128 * tt : 128 * (tt + 1)],
                                    in0=pd[:, 128 * j : 128 * (j + 1)],
                                    in1=Bb(f"R_{l}"), op=Alu.add,
                                )
                            else:
                                # u_out kept in fp16 tmp until scal is ready
                                nc.vector.tensor_tensor(
                                    out=tmp16_sb[:, 128 * tt : 128 * (tt + 1)],
                                    in0=pd[:, 128 * j : 128 * (j + 1)],
                                    in1=Bb(f"u_{l}"), op=Alu.add,
                                )

                # di update (full width, fp16 2x)
                def k4(sb, k):
                    a = sb[:]
                    return bass.AP(tensor=a.tensor, offset=a.offset + 128 * k,
                                   ap=[a.ap[0], [384, NT], [1, 128]])
                R3 = bass.AP(tensor=Rout_sb[:].tensor, offset=Rout_sb[:].offset,
                             ap=[Rout_sb[:].ap[0], [128, NT], [1, 128]])
                s3 = bass.AP(tensor=scal_sb[:].tensor, offset=scal_sb[:].offset,
                             ap=[scal_sb[:].ap[0], [128, NT], [1, 128]])
                if l == 0:
                    fi4 = bass.AP(tensor=fi_sb[:].tensor, offset=fi_sb[:].offset,
                                  ap=[fi_sb[:].ap[0], [384, NT], [128, 3], [1, 128]])
                    di4 = bass.AP(tensor=di_sb[:].tensor, offset=di_sb[:].offset,
                                  ap=[di_sb[:].ap[0], [384, NT], [128, 3], [1, 128]])
                    R4 = bass.AP(tensor=Rout_sb[:].tensor, offset=Rout_sb[:].offset,
                                 ap=[Rout_sb[:].ap[0], [128, NT], [0, 3], [1, 128]])
                    nc.vector.tensor_tensor(out=di4, in0=fi4, in1=R4, op=Alu.mult)
                else:
                    phi3 = bass.AP(tensor=phi_sb[:].tensor, offset=phi_sb[:].offset,
                                   ap=[phi_sb[:].ap[0], [128, NT], [1, 128]])
                    for k in range(3):
                        fk, dk = k4(fi_sb, k), k4(di_sb, k)
                        nc.vector.tensor_tensor(out=s3, in0=fk, in1=R3, op=Alu.mult)
                        nc.vector.tensor_tensor(out=dk, in0=dk, in1=phi3, op=Alu.mult)
                        nc.vector.tensor_tensor(out=dk, in0=dk, in1=s3, op=Alu.add)
                # scal = sum_k fi_k * di_k  (into scal_sb, tmp in Rout? no - use
                # scal + tmp16 is occupied by u_out; reuse fvs? sizes differ).
                # Use: scal = fi0*di0; scal += fi1*di1 via two-step with xi16 as
                # scratch (xi16 is stale until next transpose_pass).
                x16 = bass.AP(tensor=xi16_sb[:].tensor, offset=xi16_sb[:].offset,
                              ap=[xi16_sb[:].ap[0], [128, NT], [1, 128]])
                nc.vector.tensor_tensor(out=s3, in0=k4(fi_sb, 0), in1=k4(di_sb, 0), op=Alu.mult)
                nc.vector.tensor_tensor(out=x16, in0=k4(fi_sb, 1), in1=k4(di_sb, 1), op=Alu.mult)
                nc.vector.tensor_tensor(out=s3, in0=s3, in1=x16, op=Alu.add)
                nc.vector.tensor_tensor(out=x16, in0=k4(fi_sb, 2), in1=k4(di_sb, 2), op=Alu.mult)
                nc.vector.tensor_tensor(out=s3, in0=s3, in1=x16, op=Alu.add)
                # xi -= u_out * scal
                t3 = bass.AP(tensor=tmp16_sb[:].tensor, offset=tmp16_sb[:].offset,
                             ap=[tmp16_sb[:].ap[0], [128, NT], [1, 128]])
                nc.vector.tensor_tensor(out=t3, in0=t3, in1=s3, op=Alu.mult)
                nc.vector.tensor_tensor(out=xi_sb[:], in0=xi_sb[:], in1=tmp16_sb[:], op=Alu.subtract)

            # ---- output ----
            nc.sync.dma_start(
                xi_out_d[:].rearrange("(t p) c -> p t c", p=128),
                xi_sb[:].rearrange("p (t c) -> p t c", t=NT),
            )

    nc.compile()
    return nc


# ---------------------------------------------------------------------------
# Entry point
# ---------------------------------------------------------------------------

_CACHE = {}


def kernel(**inputs):
    from concourse.bass_utils import run_bass_kernel_spmd

    meta, per_core = host_prep(inputs)
    wts, widx, bcols, bbc, bidx, spw = pack_weights(inputs)

    key = (meta["NT"], meta["B"], tuple(meta["R"]))
    if key not in _CACHE:
        _CACHE[key] = build_nc(meta, widx, bidx, wts.shape[0], bbc.shape[0])
    nc = _CACHE[key]

    in_maps = []
    for ci, pc in enumerate(per_core):
        in_maps.append(
            dict(
                wts=wts, spw=spw, bcols=bcols, bbc=bbc,
                rbsw=pc["rbsw"], dsti=pc["dsti_w"], spi=pc["spi_w"],
                swcol=pc["swcol"], dirsw=pc["dirsw"], swsum=pc["swsum"],
            )
        )
    res = run_bass_kernel_spmd(nc, in_maps, core_ids=list(range(N_CORES)))

    N = np.asarray(inputs["species"]).shape[0]
    out = np.zeros((N, DIM), np.float32)
    for ci, pc in enumerate(per_core):
        out[pc["perm"]] = res.results[ci]["xi_out"][: pc["n_own"]]
    return out


# revision 20
# speedup vs baseline: 1.0356x; 1.0356x over previous
"""NewtonNet embedding kernel for 8 TRN2 NeuronCores (Bass/Tile).

Strategy (graph/data parallel on edges, per the sharding hint):
  - Edges (sorted by src) are sharded 8 ways at segment boundaries; each core
    owns the contiguous node range its edges cover.  Within a core, owned
    nodes are sorted by degree; the edge for node-slot `128*t + p` (round r)
    sits at partition p of edge block (tile t, round r), so segment sums are
    per-partition accumulates and the src-side gather is the identity.
  - All matmul operands are fp16 (1 cy/row on the PE vs 4 for fp32) with fp32
    PSUM accumulation.  The edge pipeline runs transposed ([c, e] layout):
    DijT comes straight out of the radial matmul, the dst-gather returns
    transposed rows, and mijT feeds the edge MLPs with no PE transpose.
  - Per-layer a-MLP output is AllGathered (fp16) and parked in SBUF in the
    dma_gather SBUF-source stripe layout; dst rows are fetched with
    transposed SBUF-source gathers (256B elements).
  - xi segment-sum runs on the PE (mijT.T @ I accumulated in PSUM per tile).
  - fi/phi segment sums are per-partition scalar*tensor accumulates split
    between DVE (stt) and ACT (scaled copy) + DVE add.
"""

import os
import sys

import numpy as np

sys.path.insert(0, "/opt/trn_rl_repo")

N_CORES = 8
DIM = 128
NLAYERS = 3
NBASIS = 16
CUTOFF = 5.0
CHUNK_BLOCKS = 8  # edge blocks per indirect_copy chunk

# ---------------------------------------------------------------------------
# Host-side preprocessing
# ---------------------------------------------------------------------------


def host_prep(inputs):
    species = np.asarray(inputs["species"])
    src = np.asarray(inputs["edge_src"])
    dst = np.asarray(inputs["edge_dst"])
    d = np.asarray(inputs["distances"]).astype(np.float32)
    vec = np.asarray(inputs["vec"]).astype(np.float32)
    sw = np.asarray(inputs["switch"]).astype(np.float32)
    E = src.shape[0]
    N = species.shape[0]

    cuts = [0]
    for c in range(1, N_CORES):
        k = c * E // N_CORES
        while k < E and src[k] == src[k - 1]:
            k += 1
        cuts.append(k)
    cuts.append(E)
    nb = [0]
    for c in range(1, N_CORES):
        nb.append(int(src[cuts[c]]) if cuts[c] < E else N)
    nb.append(N)

    deg = np.bincount(src, minlength=N)
    cores = []
    for c in range(N_CORES):
        lo, hi = nb[c], nb[c + 1]
        nodes = np.arange(lo, hi)
        order = np.argsort(-deg[lo:hi], kind="stable")
        perm = nodes[order]
        cores.append(dict(perm=perm, deg=deg[perm]))

    NT = max((len(c["perm"]) + 127) // 128 for c in cores)
    n_pad = NT * 128

    R = []
    for t in range(NT):
        r = 0
        for c in cores:
            dg = c["deg"]
            if 128 * t < len(dg):
                r = max(r, int(dg[128 * t]))
        R.append(max(r, 1))
    B = sum(R)
    E_dev = 128 * B

    owner = np.zeros(N, np.int32)
    slot = np.zeros(N, np.int32)
    for ci, c in enumerate(cores):
        owner[c["perm"]] = ci
        slot[c["perm"]] = np.arange(len(c["perm"]))

    centers = np.linspace(0.0, CUTOFF, NBASIS).astype(np.float32)
    eta = np.float32((NBASIS / CUTOFF) ** 2)
    estart = np.zeros(N + 1, np.int64)
    np.cumsum(deg, out=estart[1:])

    per_core = []
    for ci, c in enumerate(cores):
        perm, dgs = c["perm"], c["deg"]
        n_own = len(perm)
        # device edge id for (slot, round) -> global edge index (or -1 dummy)
        eid = np.full(E_dev, -1, np.int64)
        off = 0
        for t in range(NT):
            s0 = 128 * t
            s1 = min(s0 + 128, n_own)
            if s1 > s0:
                pp = np.arange(s0, s1)
                dg_t = dgs[pp]
                base = estart[perm[pp]]
                for r in range(R[t]):
                    b = off + r
                    m = r < dg_t
                    idx = 128 * b + (pp - s0)
                    eid[idx[m]] = base[m] + r
            off += R[t]
        valid = eid >= 0
        ge = np.where(valid, eid, 0)

        de = d[ge]
        swe = np.where(valid, sw[ge], 0.0).astype(np.float32)
        rbsw = np.zeros((17, E_dev), np.float16)
        rbsw[:16] = (
            np.exp(-eta * (de[None, :] - centers[:, None]) ** 2) * swe[None, :]
        ).astype(np.float16)
        rbsw[16] = swe.astype(np.float16)

        dire = vec[ge] / de[:, None] * swe[:, None]

        dsti = (owner[dst[ge]].astype(np.int64) * n_pad + slot[dst[ge]]).astype(
            np.uint16
        )
        dsti[~valid] = 0

        sw_dev = swe.reshape(B, 128)
        swsum = np.zeros((128, NT), np.float32)
        o = 0
        for t in range(NT):
            swsum[:, t] = sw_dev[o : o + R[t]].sum(axis=0)
            o += R[t]

        swcol = np.ascontiguousarray(sw_dev.T)  # [128, B]
        dirsw = np.ascontiguousarray(
            dire.reshape(B, 128, 3).transpose(1, 0, 2).reshape(128, B * 3)
        )
        dsti_w = np.tile(dsti.reshape(-1, 16).T, (8, 1))  # replicated for 8 Q7 cores
        spi = np.zeros(n_pad, np.int16)
        spi[:n_own] = species[perm]
        spi_w = np.tile(spi.reshape(-1, 16).T, (8, 1))

        per_core.append(
            dict(
                rbsw=rbsw,
                dirsw=dirsw,
                swcol=swcol,
                swsum=swsum,
                dsti_w=dsti_w,
                spi_w=spi_w,
                perm=perm,
                n_own=n_own,
            )
        )

    meta = dict(NT=NT, n_pad=n_pad, R=R, B=B, E_dev=E_dev)
    return meta, per_core


def pack_weights(inputs):
    """Pack weight matrices into one fp16 [NW,128,128] tensor + bias tensors."""
    W = {k: np.asarray(v).astype(np.float32) for k, v in inputs.items()}
    mats = []
    widx = {}

    def add(name, m):
        widx[name] = len(mats)
        p = np.zeros((128, 128), np.float16)
        p[: m.shape[0], : m.shape[1]] = m.astype(np.float16)
        mats.append(p)

    for l in range(NLAYERS):
        for nm in ["a", "F", "f", "r", "R", "u"]:
            add(f"{nm}_W1_{l}", W[f"{nm}_W1"][l])
            add(f"{nm}_W2_{l}", W[f"{nm}_W2"][l])
        add(
            f"radial_Wb_{l}",
            np.concatenate([W["radial_W"][l], W["radial_b"][l][None]], 0),
        )
    spw = np.zeros((128, 128), np.float32)
    spw[:100] = W["species_W"]
    wts = np.stack(mats)  # [NW,128,128] fp16

    # bias columns [128, NC] fp32:
    #   cols 6l+j          : b1 of (a,F,f,r,R,u) layer l
    #   cols 18+l          : F_b2 layer l (broadcast scalar)
    #   cols 21+l          : a_b2 layer l
    bcols = np.zeros((128, 6 * NLAYERS + 2 * NLAYERS), np.float32)
    for l in range(NLAYERS):
        for j, nm in enumerate(["a", "F", "f", "r", "R", "u"]):
            bcols[:, 6 * l + j] = W[f"{nm}_b1"][l]
        bcols[:, 6 * NLAYERS + l] = W["F_b2"][l][0]
        bcols[:, 7 * NLAYERS + l] = W["a_b2"][l]

    # broadcast b2 tiles [NB,128,128] fp16: 0=species_b; per-layer f,r,R,u
    bidx = {"species": 0}
    bl = [np.tile(W["species_b"][None], (128, 1)).astype(np.float16)]
    for nm in ["f", "r", "R", "u"]:
        for l in range(NLAYERS):
            bidx[f"{nm}_{l}"] = len(bl)
            bl.append(np.tile(W[f"{nm}_b2"][l][None], (128, 1)).astype(np.float16))
    bbc = np.stack(bl)
    return wts, widx, bcols, bbc, bidx, spw


# ---------------------------------------------------------------------------
# Device kernel builder
# ---------------------------------------------------------------------------


def build_nc(meta, widx, bidx, NW, NB_bbc, no_cc=False, no_gather=False, nlayers=NLAYERS):
    import concourse.bass as bass
    import concourse.bacc as bacc
    import concourse.mybir as mybir
    import concourse.tile as tile
    from concourse.masks import make_identity

    NT, n_pad, R, B, E_dev = meta["NT"], meta["n_pad"], meta["R"], meta["B"], meta["E_dev"]
    NS = N_CORES * n_pad // 128  # stripes in the SBUF gather source
    f32 = mybir.dt.float32
    f16 = mybir.dt.float16
    i16 = mybir.dt.int16
    Alu = mybir.AluOpType
    Act = mybir.ActivationFunctionType

    nc = bacc.Bacc("TRN2", target_bir_lowering=False, debug=False, num_devices=N_CORES)

    # ---- I/O ----
    wts_d = nc.dram_tensor("wts", [NW, 128, 128], f16, kind="ExternalInput")
    spw_d = nc.dram_tensor("spw", [128, 128], f32, kind="ExternalInput")
    bcols_d = nc.dram_tensor("bcols", [128, 8 * NLAYERS], f32, kind="ExternalInput")
    bbc_d = nc.dram_tensor("bbc", [NB_bbc, 128, 128], f16, kind="ExternalInput")
    rbsw_d = nc.dram_tensor("rbsw", [17, E_dev], f16, kind="ExternalInput")
    dsti_d = nc.dram_tensor("dsti", [128, E_dev // 16], mybir.dt.uint16, kind="ExternalInput")
    spi_d = nc.dram_tensor("spi", [128, n_pad // 16], i16, kind="ExternalInput")
    swcol_d = nc.dram_tensor("swcol", [128, B], f32, kind="ExternalInput")
    dirsw_d = nc.dram_tensor("dirsw", [128, 3 * B], f32, kind="ExternalInput")
    swsum_d = nc.dram_tensor("swsum", [128, NT], f32, kind="ExternalInput")
    xi_out_d = nc.dram_tensor("xi_out", [n_pad, 128], f32, kind="ExternalOutput")

    ai_bounce = nc.dram_tensor("ai_bounce", [128, n_pad], f16)
    ai_all_sh = nc.dram_tensor("ai_all_sh", [N_CORES * 128, n_pad], f16, addr_space="Shared")

    # ---- persistent SBUF ----
    wts_sb = nc.alloc_sbuf_tensor("wts_sb", [128, NW * 128], f16)
    bcols_sb = nc.alloc_sbuf_tensor("bcols_sb", [128, 8 * NLAYERS], f32)
    bbc_sb = nc.alloc_sbuf_tensor("bbc_sb", [128, NB_bbc * 128], f16)
    swcol_sb = nc.alloc_sbuf_tensor("swcol_sb", [128, B], f32)
    dirsw_sb = nc.alloc_sbuf_tensor("dirsw_sb", [128, 3 * B], f32)
    swsum_sb = nc.alloc_sbuf_tensor("swsum_sb", [128, NT], f32)
    dsti_sb = nc.alloc_sbuf_tensor("dsti_sb", [128, E_dev // 16], mybir.dt.uint16)
    spi_sb = nc.alloc_sbuf_tensor("spi_sb", [128, n_pad // 16], i16)
    ident_sb = nc.alloc_sbuf_tensor("ident_sb", [128, 128], f16)

    xi_sb = nc.alloc_sbuf_tensor("xi_sb", [128, NT * 128], f32)    # node-major state
    xi16_sb = nc.alloc_sbuf_tensor("xi16_sb", [128, NT * 128], f16)
    xiT_sb = nc.alloc_sbuf_tensor("xiT_sb", [128, n_pad], f16)     # [c, node]
    aiT_sb = nc.alloc_sbuf_tensor("aiT_sb", [128, n_pad], f16)     # [c, node]
    aiallT_sb = nc.alloc_sbuf_tensor("aiallT_sb", [128, N_CORES * n_pad], f16)  # [c, global node]
    fi_sb = nc.alloc_sbuf_tensor("fi_sb", [128, NT * 3 * 128], f16)
    di_sb = nc.alloc_sbuf_tensor("di_sb", [128, NT * 3 * 128], f16)
    phi_sb = nc.alloc_sbuf_tensor("phi_sb", [128, NT * 128], f16)
    Rout_sb = nc.alloc_sbuf_tensor("Rout_sb", [128, NT * 128], f16)
    scal_sb = nc.alloc_sbuf_tensor("scal_sb", [128, NT * 128], f16)
    tmp16_sb = nc.alloc_sbuf_tensor("tmp16_sb", [128, NT * 128], f16)
    fvs_sb = nc.alloc_sbuf_tensor("fvs_sb", [128, 12], f32)
    fvs3_sb = nc.alloc_sbuf_tensor("fvs3_sb", [128, 3], f32)

    def W(name):
        m = widx[name]
        return wts_sb[:, 128 * m : 128 * (m + 1)]

    def Bb(name):
        m = bidx[name]
        return bbc_sb[:, 128 * m : 128 * (m + 1)]

    def b1col(l, nm):
        j = 6 * l + ["a", "F", "f", "r", "R", "u"].index(nm)
        return bcols_sb[:, j : j + 1]

    def ap3(ap2d, base, gw):
        """[128, gw, 128] view of contiguous cols [base, base+gw*128)."""
        a = ap2d[:, base : base + gw * 128]
        return bass.AP(
            tensor=a.tensor, offset=a.offset, ap=[a.ap[0], [128, gw], [1, 128]]
        )

    def rep3(ap2d, base, gw):
        """[128, gw, 128] broadcast of cols [base, base+128) repeated gw times."""
        a = ap2d[:, base : base + 128]
        return bass.AP(tensor=a.tensor, offset=a.offset, ap=[a.ap[0], [0, gw], [1, 128]])

    # tile index of each block, block ranges per tile
    tile_off = []
    o = 0
    for t in range(NT):
        tile_off.append(o)
        o += R[t]
    n_chunks = (B + CHUNK_BLOCKS - 1) // CHUNK_BLOCKS

    with tile.TileContext(nc) as tc:
        with (
            tc.tile_pool(name="psA", bufs=2, space="PSUM") as psA,  # DijT / transposes
            tc.tile_pool(name="psC", bufs=2, space="PSUM") as psC,  # mm1 h1
            tc.tile_pool(name="psD", bufs=2, space="PSUM") as psD,  # mm2 outs
            tc.tile_pool(name="psX", bufs=1, space="PSUM") as psX,  # xi accum
            tc.tile_pool(name="sbg", bufs=3) as sbg,  # adT gather chunks
            tc.tile_pool(name="sbr", bufs=3) as sbr,  # rbsw chunks
            tc.tile_pool(name="sbt", bufs=2) as sbt,  # tmp = adT*aiT
            tc.tile_pool(name="sbm", bufs=2) as sbm,  # mijT
            tc.tile_pool(name="sbh", bufs=4) as sbh,  # h1s tiles
            tc.tile_pool(name="sbp", bufs=3) as sbp,  # ACT fi products
            tc.tile_pool(name="sbv", bufs=2) as sbv,  # fv smalls
        ):
            # ---- load resident data ----
            nc.sync.dma_start(
                wts_sb[:].rearrange("p (m n) -> p m n", m=NW),
                wts_d[:].rearrange("m p n -> p m n"),
            )
            nc.sync.dma_start(bcols_sb[:], bcols_d[:])
            nc.sync.dma_start(
                bbc_sb[:].rearrange("p (m n) -> p m n", m=NB_bbc),
                bbc_d[:].rearrange("m p n -> p m n"),
            )
            nc.sync.dma_start(swcol_sb[:], swcol_d[:])
            nc.sync.dma_start(dirsw_sb[:], dirsw_d[:])
            nc.sync.dma_start(swsum_sb[:], swsum_d[:])
            nc.sync.dma_start(dsti_sb[:], dsti_d[:])
            nc.sync.dma_start(spi_sb[:], spi_d[:])
            make_identity(nc, ident_sb[:])

            # ---- xi0 = species_W[species] + species_b ----
            for g0 in range(0, NT, 8):
                gn = min(8, NT - g0)
                nc.gpsimd.dma_gather(
                    out_ap=xi_sb[:, 128 * g0 : 128 * (g0 + gn)].rearrange(
                        "p (t c) -> p t c", t=gn
                    ),
                    in_ap=spw_d[:],
                    idxs_ap=spi_sb[:, 8 * g0 : 8 * (g0 + gn)],
                    num_idxs=128 * gn,
                    num_idxs_reg=128 * gn,
                    elem_size=128,
                )
            nc.vector.tensor_tensor(
                out=ap3(xi_sb[:], 0, NT),
                in0=ap3(xi_sb[:], 0, NT),
                in1=rep3(bbc_sb[:], 128 * bidx["species"], NT),
                op=Alu.add,
            )
            nc.vector.memset(fi_sb[:], 0.0)

            def transpose_pass():
                """xi (f32 node-major) -> xi16 -> xiT (f16 [c, node])."""
                nc.scalar.copy(out=xi16_sb[:], in_=xi_sb[:])
                for t in range(NT):
                    ps = psC.tile([128, 512], f16, tag="h1")
                    nc.tensor.transpose(
                        out=ps[:, 0:128],
                        in_=xi16_sb[:, 128 * t : 128 * (t + 1)],
                        identity=ident_sb[:],
                    )
                    nc.scalar.copy(
                        out=xiT_sb[:, 128 * t : 128 * (t + 1)], in_=ps[:, 0:128]
                    )

            # =========================== layers ===========================
            for l in range(nlayers):
                transpose_pass()

                # ---- a-MLP (wide, transposed output) ----
                for c0 in range(0, n_pad, 512):
                    w = min(512, n_pad - c0)
                    h1p = psC.tile([128, 512], f32, tag="h1")
                    nc.tensor.matmul(
                        h1p[:, :w], lhsT=W(f"a_W1_{l}"), rhs=xiT_sb[:, c0 : c0 + w],
                        start=True, stop=True,
                    )
                    h1s = sbh.tile([128, 512], f16, tag="h1s")
                    nc.scalar.activation(
                        out=h1s[:, :w], in_=h1p[:, :w], func=Act.Silu, bias=b1col(l, "a")
                    )
                    pa = psD.tile([128, 512], f32, tag="mm2")
                    nc.tensor.matmul(
                        pa[:, :w], lhsT=W(f"a_W2_{l}"), rhs=h1s[:, :w],
                        start=True, stop=True,
                    )
                    nc.scalar.add(
                        out=aiT_sb[:, c0 : c0 + w], in_=pa[:, :w],
                        add=bcols_sb[:, 7 * NLAYERS + l : 7 * NLAYERS + l + 1],
                    )
                nc.sync.dma_start(ai_bounce[:], aiT_sb[:])
                if no_cc:
                    for _ci in range(N_CORES):
                        nc.sync.dma_start(
                            ai_all_sh[_ci * 128 : (_ci + 1) * 128, :], ai_bounce[:]
                        )
                else:
                    nc.gpsimd.collective_compute(
                        "AllGather",
                        Alu.bypass,
                        replica_groups=[list(range(N_CORES))],
                        ins=[ai_bounce.ap().opt()],
                        outs=[ai_all_sh.ap().opt()],
                    )
                nc.sync.dma_start(
                    aiallT_sb[:].rearrange("p (g n) -> p g n", g=N_CORES),
                    ai_all_sh[:].rearrange("(g p) n -> p g n", p=128),
                )
                if l > 0:
                    nc.vector.memset(phi_sb[:], 0.0)

                # ---- edge phase ----
                chunk_tiles = {}

                def issue_chunk(ci):
                    b0 = ci * CHUNK_BLOCKS
                    nb_ = min(CHUNK_BLOCKS, B - b0)
                    ad = sbg.tile([128, CHUNK_BLOCKS * 128], f16, tag="ad")
                    if no_gather:
                        nc.vector.memset(ad[:], 1.0)
                    else:
                        nc.gpsimd.indirect_copy(
                            out=ad[:, : nb_ * 128].rearrange(
                                "p (e o) -> p e o", o=1
                            ),
                            data=aiallT_sb[:].rearrange("p (n o) -> p n o", o=1),
                            idxs=dsti_sb[:, 8 * b0 : 8 * (b0 + nb_)],
                            i_know_ap_gather_is_preferred=True,
                        )
                    rb = sbr.tile([17, CHUNK_BLOCKS * 128], f16, tag="rbsw")
                    nc.sync.dma_start(
                        rb[:, : nb_ * 128], rbsw_d[:, 128 * b0 : 128 * (b0 + nb_)]
                    )
                    chunk_tiles[ci] = (ad, rb, b0)

                def get_chunk(ci):
                    if ci not in chunk_tiles:
                        issue_chunk(ci)
                    if ci + 1 < n_chunks and (ci + 1) not in chunk_tiles:
                        issue_chunk(ci + 1)
                    return chunk_tiles[ci]

                # ---- two-stage pipelined group loop ----
                groups = []
                for t in range(NT):
                    r0 = 0
                    while r0 < R[t]:
                        b0 = tile_off[t] + r0
                        gw = min(4, R[t] - r0, CHUNK_BLOCKS - b0 % CHUNK_BLOCKS)
                        groups.append((t, b0, gw, r0 == 0, r0 + gw >= R[t]))
                        r0 += gw
                mlps = ["F", "f"] + (["r"] if l > 0 else [])
                st = {}
                tile_ps = {}

                def FRONT(gi):
                    t, b0, gw, first, last = groups[gi]
                    ci = b0 // CHUNK_BLOCKS
                    ad, rb, cb0 = get_chunk(ci)
                    w = gw * 128
                    boff = (b0 - cb0) * 128
                    # DijT [c, e] straight from the radial matmul
                    dij = psA.tile([128, 512], f32, tag="dij")
                    nc.tensor.matmul(
                        dij[:, :w], lhsT=W(f"radial_Wb_{l}")[:17, :],
                        rhs=rb[:17, boff : boff + w], start=True, stop=True,
                    )
                    # tmpT = adT * aiT(bcast over blocks)  (DVE 2x fp16)
                    tmpt = sbt.tile([128, 512], f16, tag="tmp")
                    nc.vector.tensor_tensor(
                        out=ap3(tmpt[:], 0, gw), in0=ap3(ad[:], boff, gw),
                        in1=rep3(aiT_sb[:], 128 * t, gw), op=Alu.mult,
                    )
                    # DijT PSUM -> SBUF fp16 (ACT)
                    sdij = sbt.tile([128, 512], f16, tag="sdij")
                    nc.scalar.copy(out=sdij[:, :w], in_=dij[:, :w])
                    # mijT = tmpT * DijT  (DVE 2x fp16)
                    mijT = sbm.tile([128, 512], f16, tag="mijT", bufs=3)
                    nc.vector.tensor_tensor(
                        out=mijT[:, :w], in0=tmpt[:, :w], in1=sdij[:, :w],
                        op=Alu.mult,
                    )
                    st[gi] = mijT

                def BACK(gi):
                    t, b0, gw, first, last = groups[gi]
                    w = gw * 128
                    mijT = st.pop(gi)
                    if first:
                        tile_ps[t] = psX.tile(
                            [128, 128], f32, tag="xiacc", bufs=1, name=f"xiacc{l}_{t}"
                        )
                        nc.vector.memset(fvs_sb[:], 0.0)
                    psxt = tile_ps[t]
                    # xi segment sum on PE: psxt += mijT_j.T @ I
                    for j in range(gw):
                        nc.tensor.matmul(
                            psxt[:, :], lhsT=mijT[:, 128 * j : 128 * (j + 1)],
                            rhs=ident_sb[:],
                            start=first and j == 0, stop=last and j == gw - 1,
                            skip_group_check=True,
                        )
                    # edge MLP hidden layers
                    h1 = {}
                    for nm in mlps:
                        hp = psC.tile([128, 512], f32, tag="h1")
                        nc.tensor.matmul(
                            hp[:, :w], lhsT=W(f"{nm}_W1_{l}"), rhs=mijT[:, :w],
                            start=True, stop=True,
                        )
                        hs = sbh.tile([128, 512], f16, tag="h1s")
                        nc.scalar.activation(
                            out=hs[:, :w], in_=hp[:, :w], func=Act.Silu,
                            bias=b1col(l, nm),
                        )
                        h1[nm] = hs

                    # F head: one column per block
                    pf = psD.tile([128, 4], f32, tag="mm2")
                    for j in range(gw):
                        nc.tensor.matmul(
                            pf[:, j : j + 1],
                            lhsT=h1["F"][:, 128 * j : 128 * (j + 1)],
                            rhs=W(f"F_W2_{l}")[:, 0:1],
                            start=True, stop=True,
                        )
                    # fv = (F + F_b2) * dirsw    [128, gw*3], block-major
                    fvb = sbv.tile([128, 4], f32, tag="fvb")
                    nc.vector.tensor_scalar(
                        out=fvb[:, :gw], in0=pf[:, :gw],
                        scalar1=bcols_sb[:, 6 * NLAYERS + l : 6 * NLAYERS + l + 1],
                        scalar2=None, op0=Alu.add,
                    )
                    fv12 = sbv.tile([128, 12], f32, tag="fv12")
                    d0 = dirsw_sb[:, 3 * b0 : 3 * (b0 + gw)]
                    nc.vector.tensor_tensor(
                        out=bass.AP(tensor=fv12[:].tensor, offset=fv12[:].offset,
                                    ap=[fv12[:].ap[0], [3, gw], [1, 3]]),
                        in0=bass.AP(tensor=d0.tensor, offset=d0.offset,
                                    ap=[d0.ap[0], [3, gw], [1, 3]]),
                        in1=bass.AP(tensor=fvb[:].tensor, offset=fvb[:].offset,
                                    ap=[fvb[:].ap[0], [1, gw], [0, 3]]),
                        op=Alu.mult,
                    )
                    nc.vector.tensor_tensor(
                        out=fvs_sb[:, : 3 * gw], in0=fvs_sb[:, : 3 * gw],
                        in1=fv12[:, : 3 * gw], op=Alu.add,
                    )

                    # f head + fi accumulation
                    pd_f = psD.tile([128, 512], f32, tag="mm2")
                    for j in range(gw):
                        nc.tensor.matmul(
                            pd_f[:, 128 * j : 128 * (j + 1)],
                            lhsT=h1["f"][:, 128 * j : 128 * (j + 1)],
                            rhs=W(f"f_W2_{l}"),
                            start=True, stop=True,
                        )
                    for j in range(gw):
                        fo = pd_f[:, 128 * j : 128 * (j + 1)]
                        for k in range(2):
                            fslice = fi_sb[
                                :, (3 * t + k) * 128 : (3 * t + k + 1) * 128
                            ]
                            nc.vector.scalar_tensor_tensor(
                                out=fslice, in0=fo,
                                scalar=fv12[:, 3 * j + k : 3 * j + k + 1],
                                in1=fslice, op0=Alu.mult, op1=Alu.add,
                            )
                        # k=2 via ACT scaled copy + DVE add
                        prod = sbp.tile([128, 128], f16, tag="prod")
                        nc.scalar.activation(
                            out=prod[:], in_=fo, func=Act.Copy,
                            scale=fv12[:, 3 * j + 2 : 3 * j + 3],
                        )
                        fslice = fi_sb[:, (3 * t + 2) * 128 : (3 * t + 3) * 128]
                        nc.vector.tensor_tensor(
                            out=fslice, in0=fslice, in1=prod[:], op=Alu.add
                        )

                    # r head + phi accumulation
                    if l > 0:
                        pd_r = psD.tile([128, 512], f32, tag="mm2")
                        for j in range(gw):
                            nc.tensor.matmul(
                                pd_r[:, 128 * j : 128 * (j + 1)],
                                lhsT=h1["r"][:, 128 * j : 128 * (j + 1)],
                                rhs=W(f"r_W2_{l}"),
                                start=True, stop=True,
                            )
                        pslice = phi_sb[:, 128 * t : 128 * (t + 1)]
                        for j in range(gw):
                            nc.vector.scalar_tensor_tensor(
                                out=pslice,
                                in0=pd_r[:, 128 * j : 128 * (j + 1)],
                                scalar=swcol_sb[:, b0 + j : b0 + j + 1],
                                in1=pslice, op0=Alu.mult, op1=Alu.add,
                            )

                    if last:
                        # ---- tile end folds ----
                        xt = xi_sb[:, 128 * t : 128 * (t + 1)]
                        nc.vector.tensor_tensor(out=xt, in0=xt, in1=psxt[:, :], op=Alu.add)
                        nc.vector.tensor_reduce(
                            out=fvs3_sb[:],
                            in_=bass.AP(tensor=fvs_sb[:].tensor, offset=fvs_sb[:].offset,
                                        ap=[fvs_sb[:].ap[0], [1, 3], [3, 4]]),
                            axis=mybir.AxisListType.X,
                            op=Alu.add,
                        )
                        for k in range(3):
                            fslice = fi_sb[:, (3 * t + k) * 128 : (3 * t + k + 1) * 128]
                            nc.vector.scalar_tensor_tensor(
                                out=fslice, in0=Bb(f"f_{l}"), scalar=fvs3_sb[:, k : k + 1],
                                in1=fslice, op0=Alu.mult, op1=Alu.add,
                            )
                        if l > 0:
                            pslice = phi_sb[:, 128 * t : 128 * (t + 1)]
                            nc.vector.scalar_tensor_tensor(
                                out=pslice, in0=Bb(f"r_{l}"), scalar=swsum_sb[:, t : t + 1],
                                in1=pslice, op0=Alu.mult, op1=Alu.add,
                            )
                        del tile_ps[t]

                for gi in range(len(groups) + 1):
                    if gi < len(groups):
                        FRONT(gi)
                    if gi >= 1:
                        BACK(gi - 1)

                # ---- node phase ----
                transpose_pass()  # xi is now xi_mid

                # R-MLP and u-MLP (hidden wide; mm2 per tile, node-major out)
                pd_u = {}
                for nm, dest in (("R", "Rout"), ("u", None)):
                    for c0 in range(0, n_pad, 512):
                        w = min(512, n_pad - c0)
                        h1p = psC.tile([128, 512], f32, tag="h1")
                        nc.tensor.matmul(
                            h1p[:, :w], lhsT=W(f"{nm}_W1_{l}"),
                            rhs=xiT_sb[:, c0 : c0 + w], start=True, stop=True,
                        )
                        h1s = sbh.tile([128, 512], f16, tag="h1s")
                        nc.scalar.activation(
                            out=h1s[:, :w], in_=h1p[:, :w], func=Act.Silu,
                            bias=b1col(l, nm),
                        )
                        pd = psD.tile([128, 512], f32, tag="mm2")
                        for j in range(w // 128):
                            nc.tensor.matmul(
                                pd[:, 128 * j : 128 * (j + 1)],
                                lhsT=h1s[:, 128 * j : 128 * (j + 1)],
                                rhs=W(f"{nm}_W2_{l}"),
                                start=True, stop=True,
                            )
                        for j in range(w // 128):
                            tt = (c0 + 128 * j) // 128
                            if dest == "Rout":
                                nc.vector.tensor_tensor(
                                    out=Rout_sb[:, 128 * tt : 128 * (tt + 1)],
                                    in0=pd[:, 128 * j : 128 * (j + 1)],
                                    in1=Bb(f"R_{l}"), op=Alu.add,
                                )
                            else:
                                # u_out kept in fp16 tmp until scal is ready
                                nc.vector.tensor_tensor(
                                    out=tmp16_sb[:, 128 * tt : 128 * (tt + 1)],
                                    in0=pd[:, 128 * j : 128 * (j + 1)],
                                    in1=Bb(f"u_{l}"), op=Alu.add,
                                )

                # di update (full width, fp16 2x)
                def k4(sb, k):
                    a = sb[:]
                    return bass.AP(tensor=a.tensor, offset=a.offset + 128 * k,
                                   ap=[a.ap[0], [384, NT], [1, 128]])
                R3 = bass.AP(tensor=Rout_sb[:].tensor, offset=Rout_sb[:].offset,
                             ap=[Rout_sb[:].ap[0], [128, NT], [1, 128]])
                s3 = bass.AP(tensor=scal_sb[:].tensor, offset=scal_sb[:].offset,
                             ap=[scal_sb[:].ap[0], [128, NT], [1, 128]])
                if l == 0:
                    fi4 = bass.AP(tensor=fi_sb[:].tensor, offset=fi_sb[:].offset,
                                  ap=[fi_sb[:].ap[0], [384, NT], [128, 3], [1, 128]])
                    di4 = bass.AP(tensor=di_sb[:].tensor, offset=di_sb[:].offset,
                                  ap=[di_sb[:].ap[0], [384, NT], [128, 3], [1, 128]])
                    R4 = bass.AP(tensor=Rout_sb[:].tensor, offset=Rout_sb[:].offset,
                                 ap=[Rout_sb[:].ap[0], [128, NT], [0, 3], [1, 128]])
                    nc.vector.tensor_tensor(out=di4, in0=fi4, in1=R4, op=Alu.mult)
                else:
                    phi3 = bass.AP(tensor=phi_sb[:].tensor, offset=phi_sb[:].offset,
                                   ap=[phi_sb[:].ap[0], [128, NT], [1, 128]])
                    for k in range(3):
                        fk, dk = k4(fi_sb, k), k4(di_sb, k)
                        nc.vector.tensor_tensor(out=s3, in0=fk, in1=R3, op=Alu.mult)
                        nc.vector.tensor_tensor(out=dk, in0=dk, in1=phi3, op=Alu.mult)
                        nc.vector.tensor_tensor(out=dk, in0=dk, in1=s3, op=Alu.add)
                # scal = sum_k fi_k * di_k  (into scal_sb, tmp in Rout? no - use
                # scal + tmp16 is occupied by u_out; reuse fvs? sizes differ).
                # Use: scal = fi0*di0; scal += fi1*di1 via two-step with xi16 as
                # scratch (xi16 is stale until next transpose_pass).
                x16 = bass.AP(tensor=xi16_sb[:].tensor, offset=xi16_sb[:].offset,
                              ap=[xi16_sb[:].ap[0], [128, NT], [1, 128]])
                nc.vector.tensor_tensor(out=s3, in0=k4(fi_sb, 0), in1=k4(di_sb, 0), op=Alu.mult)
                nc.vector.tensor_tensor(out=x16, in0=k4(fi_sb, 1), in1=k4(di_sb, 1), op=Alu.mult)
                nc.vector.tensor_tensor(out=s3, in0=s3, in1=x16, op=Alu.add)
                nc.vector.tensor_tensor(out=x16, in0=k4(fi_sb, 2), in1=k4(di_sb, 2), op=Alu.mult)
                nc.vector.tensor_tensor(out=s3, in0=s3, in1=x16, op=Alu.add)
                # xi -= u_out * scal
                t3 = bass.AP(tensor=tmp16_sb[:].tensor, offset=tmp16_sb[:].offset,
                             ap=[tmp16_sb[:].ap[0], [128, NT], [1, 128]])
                nc.vector.tensor_tensor(out=t3, in0=t3, in1=s3, op=Alu.mult)
                nc.vector.tensor_tensor(out=xi_sb[:], in0=xi_sb[:], in1=tmp16_sb[:], op=Alu.subtract)

            # ---- output ----
            nc.sync.dma_start(
                xi_out_d[:].rearrange("(t p) c -> p t c", p=128),
                xi_sb[:].rearrange("p (t c) -> p t c", t=NT),
            )

    nc.compile()
    return nc


# ---------------------------------------------------------------------------
# Entry point
# ---------------------------------------------------------------------------

_CACHE = {}


def kernel(**inputs):
    from concourse.bass_utils import run_bass_kernel_spmd

    meta, per_core = host_prep(inputs)
    wts, widx, bcols, bbc, bidx, spw = pack_weights(inputs)

    key = (meta["NT"], meta["B"], tuple(meta["R"]))
    if key not in _CACHE:
        _CACHE[key] = build_nc(meta, widx, bidx, wts.shape[0], bbc.shape[0])
    nc = _CACHE[key]

    in_maps = []
    for ci, pc in enumerate(per_core):
        in_maps.append(
            dict(
                wts=wts, spw=spw, bcols=bcols, bbc=bbc,
                rbsw=pc["rbsw"], dsti=pc["dsti_w"], spi=pc["spi_w"],
                swcol=pc["swcol"], dirsw=pc["dirsw"], swsum=pc["swsum"],
            )
        )
    res = run_bass_kernel_spmd(nc, in_maps, core_ids=list(range(N_CORES)))

    N = np.asarray(inputs["species"]).shape[0]
    out = np.zeros((N, DIM), np.float32)
    for ci, pc in enumerate(per_core):
        out[pc["perm"]] = res.results[ci]["xi_out"][: pc["n_own"]]
    return out
